# revision 26
# baseline (speedup 1.0000x reference)
"""Trainium2 Bass kernel for nn_CrossPatchModule.

Math (validated against the reference):
  The reference unfolds x[b,c] (512x512) into an 8x8 grid of 64x64 blocks
  (block index p = pi*8 + pj), adds pos[c, q] to block q, cyclically
  shifts blocks per channel, and folds back:

      out[b, c, block p] = x[b, c, block q] + pos[c, q],   q = (p + c) % 64

  where pos = abs_pos[0, 0, :, :, 0, 0]  (shape [64, 64], [channel, block]).

Strategy:
  - Pure data-parallel: 8 batch samples -> 8 NeuronCores (one sample each).
  - Per core, 32 tiles of two channels each, pairing c and c+32. SBUF tile:
      T[c2*64 + a, qi*512 + qj*64 + d] = x[c, qi*64 + a, qj*64 + d],
      c = i + 32*c2
    (partition = channel-half x row-within-block, free = blocks in raster
    order). The host pre-interleaves x/out into exactly this layout so
    every DMA is a dense [128, 2048] transfer with 8 KiB contiguous runs.
  - In this layout the per-channel block shift is a cyclic rotation of the
    free dim by 64*c. Channels c and c+32 need shifts that differ by
    exactly half the free dim (2048), so the host stores the c2=1 rows
    pre-rotated by 2048 (a fixed, channel-independent half-swap of its
    private layout); both halves then share one shift of 64*i and the
    fused shift+bias runs as two full-width [128, n] DVE adds per tile.
  - The per-(channel, block) bias sits compactly in SBUF ([128, 2048],
    1 MiB) and is read through a stride-0 innermost free dim
    (broadcast_to), so no on-chip broadcast pass is needed.
  - Loads/stores split across the two HWDGE rings (SP + ACT) so two
    uniform 128-partition DMAs are always in flight (all 16 SDMA engines).
"""

import os
import numpy as np

import concourse.bacc as bacc
import concourse.mybir as mybir
from concourse.tile import TileContext
from concourse.bass_utils import run_bass_kernel_spmd

B, C, H, W = 8, 64, 512, 512
PN = 64          # number of 64x64 blocks per image (8x8 grid) == C
KW = 64          # block width
FD = PN * KW     # free dim of a channel slice: 64 blocks x 64 cols = 4096
NPAIR = C // 2   # 32 channel pairs (c, c+32)
F32 = mybir.dt.float32

LAST_RESULTS = None  # BassKernelResults of the most recent run (for test.py)

_NC_CACHE = {}


def _build_nc():
    nc = bacc.Bacc("TRN2")

    x = nc.dram_tensor("x", [NPAIR, 128, FD], F32, kind="ExternalInput")
    # compact per-block bias, p-ordered:
    #   biasd[c2*64 + a, i*64 + p] = pos[c, (p + c) % 64],  c = i + 32*c2
    #   (replicated over a host-side)
    biasd = nc.dram_tensor("bias", [128, NPAIR * PN], F32, kind="ExternalInput")
    out = nc.dram_tensor("out", [NPAIR, 128, FD], F32, kind="ExternalOutput")

    with TileContext(nc) as tc:
        with (
            tc.tile_pool(name="const", bufs=1) as cpool,
            tc.tile_pool(name="io", bufs=6) as iopool,
        ):
            bias_sb = cpool.tile([128, NPAIR * PN], F32, tag="bias")
            nc.sync.dma_start(out=bias_sb[:], in_=biasd[:])

            half = FD // 2
            for i in range(NPAIR):
                tin = iopool.tile([128, FD], F32, tag="tin")
                # two uniform [128, 2048] DMAs per transfer, one per HWDGE ring
                nc.sync.dma_start(out=tin[:, 0:half], in_=x[i, :, 0:half])
                nc.scalar.dma_start(out=tin[:, half:FD], in_=x[i, :, half:FD])

                tout = iopool.tile([128, FD], F32, tag="tout")
                shift = i * KW          # shared free-dim rotation amount
                split = FD - shift      # out[f < split] <- in[f + shift]
                nblk = PN - i           # blocks in the first segment
                nc.vector.tensor_add(
                    out=tout[:, 0:split].rearrange("r (n d) -> r n d", d=KW),
                    in0=tin[:, shift:FD].rearrange("r (n d) -> r n d", d=KW),
                    in1=bias_sb[:, i * PN : i * PN + nblk][
                        :, :, None
                    ].broadcast_to([128, nblk, KW]),
                )
                if shift:
                    nc.vector.tensor_add(
                        out=tout[:, split:FD].rearrange("r (n d) -> r n d", d=KW),
                        in0=tin[:, 0:shift].rearrange("r (n d) -> r n d", d=KW),
                        in1=bias_sb[:, i * PN + nblk : (i + 1) * PN][
                            :, :, None
                        ].broadcast_to([128, i, KW]),
                    )

                nc.scalar.dma_start(out=out[i, :, 0:half], in_=tout[:, 0:half])
                nc.sync.dma_start(out=out[i, :, half:FD], in_=tout[:, half:FD])

    nc.finalize()
    return nc


def _host_bias(abs_pos: np.ndarray) -> np.ndarray:
    pos = np.asarray(abs_pos, dtype=np.float32)[0, 0, :, :, 0, 0]  # [C, PN]
    idx = (np.arange(PN)[None, :] + np.arange(C)[:, None]) % PN    # [C, p] -> q
    bias = np.take_along_axis(pos, idx, axis=1)                    # [C, p]
    # channel c = i + 32*c2 -> row block c2, column block i
    bias = bias.reshape(2, NPAIR, PN)                              # [c2, i, p]
    bias = bias.reshape(2, NPAIR * PN)                             # [c2, i*64+p]
    bias = np.repeat(bias, 64, axis=0)                             # [128, ...]
    return np.ascontiguousarray(bias)


def _interleave(xb: np.ndarray) -> np.ndarray:
    # [C, H, W] -> [NPAIR, 128, FD] tile layout; c2=1 rows pre-rotated by
    # half the free dim (qi -> (qi+4) % 8) so both halves share one shift.
    v = xb.reshape(2, NPAIR, 8, 64, 8, 64)         # (c2, i, qi, a, qj, d)
    v = np.concatenate([v[:1], np.roll(v[1:], -4, axis=2)], axis=0)
    v = v.transpose(1, 0, 3, 2, 4, 5)              # (i, c2, a, qi, qj, d)
    return np.ascontiguousarray(v.reshape(NPAIR, 128, FD))


def _deinterleave(ob: np.ndarray) -> np.ndarray:
    # [NPAIR, 128, FD] (true p-order for both halves) -> [C, H, W]
    v = ob.reshape(NPAIR, 2, 64, 8, 8, 64)         # (i, c2, a, pi, pj, d)
    v = v.transpose(1, 0, 3, 2, 4, 5)              # (c2, i, pi, a, pj, d)
    return v.reshape(C, H, W)


def kernel(x: np.ndarray, abs_pos: np.ndarray) -> np.ndarray:
    global LAST_RESULTS
    x = np.asarray(x, dtype=np.float32)
    assert x.shape == (B, C, H, W), x.shape

    bias = _host_bias(abs_pos)

    if "nc" not in _NC_CACHE:
        _NC_CACHE["nc"] = _build_nc()
    nc = _NC_CACHE["nc"]

    in_maps = [{"x": _interleave(x[b]), "bias": bias} for b in range(B)]
    res = run_bass_kernel_spmd(
        nc,
        in_maps,
        core_ids=list(range(B)),
        trace=bool(os.environ.get("KERNEL_TRACE")),
    )
    LAST_RESULTS = res
    return np.stack(
        [_deinterleave(res.results[b]["out"]) for b in range(B)], axis=0
    )


# revision 27
# speedup vs baseline: 1.0834x; 1.0834x over previous
"""Trainium2 Bass kernel for nn_CrossPatchModule.

Math (validated against the reference):
  The reference unfolds x[b,c] (512x512) into an 8x8 grid of 64x64 blocks
  (block index p = pi*8 + pj), adds pos[c, q] to block q, cyclically
  shifts blocks per channel, and folds back:

      out[b, c, block p] = x[b, c, block q] + pos[c, q],   q = (p + c) % 64

  where pos = abs_pos[0, 0, :, :, 0, 0]  (shape [64, 64], [channel, block]).

Strategy:
  - Pure data-parallel: 8 batch samples -> 8 NeuronCores (one sample each).
  - Per core, 32 tiles of two channels each, pairing c and c+32. SBUF tile:
      T[c2*64 + a, qi*512 + qj*64 + d] = x[c, qi*64 + a, qj*64 + d],
      c = i + 32*c2
    (partition = channel-half x row-within-block, free = blocks in raster
    order). The host pre-interleaves x/out into exactly this layout so
    every DMA is a dense [128, 2048] transfer with 8 KiB contiguous runs.
  - In this layout the per-channel block shift is a cyclic rotation of the
    free dim by 64*c. Channels c and c+32 need shifts that differ by
    exactly half the free dim (2048), so the host stores the c2=1 rows
    pre-rotated by 2048 (a fixed, channel-independent half-swap of its
    private layout); both halves then share one shift of 64*i and the
    fused shift+bias runs as two full-width [128, n] DVE adds per tile.
  - The per-(channel, block) bias sits compactly in SBUF ([128, 2048],
    1 MiB) and is read through a stride-0 innermost free dim
    (broadcast_to), so no on-chip broadcast pass is needed.
  - Loads/stores split across the two HWDGE rings (SP + ACT) so two
    uniform 128-partition DMAs are always in flight (all 16 SDMA engines).
"""

import os
import numpy as np

import concourse.bacc as bacc
import concourse.mybir as mybir
from concourse.tile import TileContext
from concourse.bass_utils import run_bass_kernel_spmd

B, C, H, W = 8, 64, 512, 512
PN = 64          # number of 64x64 blocks per image (8x8 grid) == C
KW = 64          # block width
FD = PN * KW     # free dim of a channel slice: 64 blocks x 64 cols = 4096
NPAIR = C // 2   # 32 channel pairs (c, c+32)
F32 = mybir.dt.float32

LAST_RESULTS = None  # BassKernelResults of the most recent run (for test.py)

_NC_CACHE = {}


def _build_nc():
    nc = bacc.Bacc("TRN2")

    x = nc.dram_tensor("x", [NPAIR, 128, FD], F32, kind="ExternalInput")
    # compact per-block bias, p-ordered:
    #   biasd[c2*64 + a, i*64 + p] = pos[c, (p + c) % 64],  c = i + 32*c2
    #   (replicated over a host-side)
    biasd = nc.dram_tensor("bias", [128, NPAIR * PN], F32, kind="ExternalInput")
    out = nc.dram_tensor("out", [NPAIR, 128, FD], F32, kind="ExternalOutput")

    with TileContext(nc) as tc:
        with (
            tc.tile_pool(name="const", bufs=1) as cpool,
            tc.tile_pool(name="io", bufs=5) as iopool,
        ):
            bias_sb = cpool.tile([128, NPAIR * PN], F32, tag="bias")
            nc.sync.dma_start(out=bias_sb[:], in_=biasd[:])

            half = FD // 2
            for i in range(NPAIR):
                tin = iopool.tile([128, FD], F32, tag="tin")
                # two uniform [128, 2048] DMAs per transfer, one per HWDGE ring
                nc.sync.dma_start(out=tin[:, 0:half], in_=x[i, :, 0:half])
                nc.scalar.dma_start(out=tin[:, half:FD], in_=x[i, :, half:FD])

                tout = iopool.tile([128, FD], F32, tag="tout")
                shift = i * KW          # shared free-dim rotation amount
                split = FD - shift      # out[f < split] <- in[f + shift]
                nblk = PN - i           # blocks in the first segment
                nc.vector.tensor_add(
                    out=tout[:, 0:split].rearrange("r (n d) -> r n d", d=KW),
                    in0=tin[:, shift:FD].rearrange("r (n d) -> r n d", d=KW),
                    in1=bias_sb[:, i * PN : i * PN + nblk][
                        :, :, None
                    ].broadcast_to([128, nblk, KW]),
                )
                if shift:
                    nc.vector.tensor_add(
                        out=tout[:, split:FD].rearrange("r (n d) -> r n d", d=KW),
                        in0=tin[:, 0:shift].rearrange("r (n d) -> r n d", d=KW),
                        in1=bias_sb[:, i * PN + nblk : (i + 1) * PN][
                            :, :, None
                        ].broadcast_to([128, i, KW]),
                    )

                nc.scalar.dma_start(out=out[i, :, 0:half], in_=tout[:, 0:half])
                nc.sync.dma_start(out=out[i, :, half:FD], in_=tout[:, half:FD])

    nc.finalize()
    return nc


def _host_bias(abs_pos: np.ndarray) -> np.ndarray:
    pos = np.asarray(abs_pos, dtype=np.float32)[0, 0, :, :, 0, 0]  # [C, PN]
    idx = (np.arange(PN)[None, :] + np.arange(C)[:, None]) % PN    # [C, p] -> q
    bias = np.take_along_axis(pos, idx, axis=1)                    # [C, p]
    # channel c = i + 32*c2 -> row block c2, column block i
    bias = bias.reshape(2, NPAIR, PN)                              # [c2, i, p]
    bias = bias.reshape(2, NPAIR * PN)                             # [c2, i*64+p]
    bias = np.repeat(bias, 64, axis=0)                             # [128, ...]
    return np.ascontiguousarray(bias)


def _interleave(xb: np.ndarray) -> np.ndarray:
    # [C, H, W] -> [NPAIR, 128, FD] tile layout; c2=1 rows pre-rotated by
    # half the free dim (qi -> (qi+4) % 8) so both halves share one shift.
    v = xb.reshape(2, NPAIR, 8, 64, 8, 64)         # (c2, i, qi, a, qj, d)
    v = np.concatenate([v[:1], np.roll(v[1:], -4, axis=2)], axis=0)
    v = v.transpose(1, 0, 3, 2, 4, 5)              # (i, c2, a, qi, qj, d)
    return np.ascontiguousarray(v.reshape(NPAIR, 128, FD))


def _deinterleave(ob: np.ndarray) -> np.ndarray:
    # [NPAIR, 128, FD] (true p-order for both halves) -> [C, H, W]
    v = ob.reshape(NPAIR, 2, 64, 8, 8, 64)         # (i, c2, a, pi, pj, d)
    v = v.transpose(1, 0, 3, 2, 4, 5)              # (c2, i, pi, a, pj, d)
    return v.reshape(C, H, W)


def kernel(x: np.ndarray, abs_pos: np.ndarray) -> np.ndarray:
    global LAST_RESULTS
    x = np.asarray(x, dtype=np.float32)
    assert x.shape == (B, C, H, W), x.shape

    bias = _host_bias(abs_pos)

    if "nc" not in _NC_CACHE:
        _NC_CACHE["nc"] = _build_nc()
    nc = _NC_CACHE["nc"]

    in_maps = [{"x": _interleave(x[b]), "bias": bias} for b in range(B)]
    res = run_bass_kernel_spmd(
        nc,
        in_maps,
        core_ids=list(range(B)),
        trace=bool(os.environ.get("KERNEL_TRACE")),
    )
    LAST_RESULTS = res
    return np.stack(
        [_deinterleave(res.results[b]["out"]) for b in range(B)], axis=0
    )


# revision 28
# speedup vs baseline: 1.1065x; 1.0213x over previous
"""Trainium2 Bass kernel for nn_CrossPatchModule.

Math (validated against the reference):
  The reference unfolds x[b,c] (512x512) into an 8x8 grid of 64x64 blocks
  (block index p = pi*8 + pj), adds pos[c, q] to block q, cyclically
  shifts blocks per channel, and folds back:

      out[b, c, block p] = x[b, c, block q] + pos[c, q],   q = (p + c) % 64

  where pos = abs_pos[0, 0, :, :, 0, 0]  (shape [64, 64], [channel, block]).

Strategy:
  - Pure data-parallel: 8 batch samples -> 8 NeuronCores (one sample each).
  - Per core, 32 tiles of two channels each, pairing c and c+32. SBUF tile:
      T[c2*64 + a, qi*512 + qj*64 + d] = x[c, qi*64 + a, qj*64 + d],
      c = i + 32*c2
    (partition = channel-half x row-within-block, free = blocks in raster
    order). The host pre-interleaves x/out into exactly this layout so
    every DMA is a dense [128, 2048] transfer with 8 KiB contiguous runs.
  - In this layout the per-channel block shift is a cyclic rotation of the
    free dim by 64*c. Channels c and c+32 need shifts that differ by
    exactly half the free dim (2048), so the host stores the c2=1 rows
    pre-rotated by 2048 (a fixed, channel-independent half-swap of its
    private layout); both halves then share one shift of 64*i and the
    fused shift+bias runs as two full-width [128, n] DVE adds per tile.
  - The per-(channel, block) bias sits compactly in SBUF ([128, 2048],
    1 MiB) and is read through a stride-0 innermost free dim
    (broadcast_to), so no on-chip broadcast pass is needed.
  - Loads/stores split across the two HWDGE rings (SP + ACT) so two
    uniform 128-partition DMAs are always in flight (all 16 SDMA engines).
"""

import os
import numpy as np

import concourse.bacc as bacc
import concourse.mybir as mybir
from concourse.tile import TileContext
from concourse.bass_utils import run_bass_kernel_spmd

B, C, H, W = 8, 64, 512, 512
PN = 64          # number of 64x64 blocks per image (8x8 grid) == C
KW = 64          # block width
FD = PN * KW     # free dim of a channel slice: 64 blocks x 64 cols = 4096
NPAIR = C // 2   # 32 channel pairs (c, c+32)
F32 = mybir.dt.float32

LAST_RESULTS = None  # BassKernelResults of the most recent run (for test.py)

_NC_CACHE = {}


def _build_nc():
    nc = bacc.Bacc("TRN2")

    x = nc.dram_tensor("x", [NPAIR, 128, FD], F32, kind="ExternalInput")
    # compact per-block bias, p-ordered:
    #   biasd[c2*64 + a, i*64 + p] = pos[c, (p + c) % 64],  c = i + 32*c2
    #   (replicated over a host-side)
    biasd = nc.dram_tensor("bias", [128, NPAIR * PN], F32, kind="ExternalInput")
    out = nc.dram_tensor("out", [NPAIR, 128, FD], F32, kind="ExternalOutput")

    with TileContext(nc) as tc:
        with (
            tc.tile_pool(name="const", bufs=1) as cpool,
            tc.tile_pool(name="io", bufs=5) as iopool,
        ):
            bias_sb = cpool.tile([128, NPAIR * PN], F32, tag="bias")
            nc.gpsimd.dma_start(out=bias_sb[:], in_=biasd[:])

            half = FD // 2
            for i in range(NPAIR):
                tin = iopool.tile([128, FD], F32, tag="tin")
                # two uniform [128, 2048] DMAs per transfer, one per HWDGE ring
                nc.sync.dma_start(out=tin[:, 0:half], in_=x[i, :, 0:half])
                nc.scalar.dma_start(out=tin[:, half:FD], in_=x[i, :, half:FD])

                tout = iopool.tile([128, FD], F32, tag="tout")
                shift = i * KW          # shared free-dim rotation amount
                split = FD - shift      # out[f < split] <- in[f + shift]
                nblk = PN - i           # blocks in the first segment
                nc.vector.tensor_add(
                    out=tout[:, 0:split].rearrange("r (n d) -> r n d", d=KW),
                    in0=tin[:, shift:FD].rearrange("r (n d) -> r n d", d=KW),
                    in1=bias_sb[:, i * PN : i * PN + nblk][
                        :, :, None
                    ].broadcast_to([128, nblk, KW]),
                )
                if shift:
                    nc.vector.tensor_add(
                        out=tout[:, split:FD].rearrange("r (n d) -> r n d", d=KW),
                        in0=tin[:, 0:shift].rearrange("r (n d) -> r n d", d=KW),
                        in1=bias_sb[:, i * PN + nblk : (i + 1) * PN][
                            :, :, None
                        ].broadcast_to([128, i, KW]),
                    )

                nc.scalar.dma_start(out=out[i, :, 0:half], in_=tout[:, 0:half])
                nc.sync.dma_start(out=out[i, :, half:FD], in_=tout[:, half:FD])

    nc.finalize()
    return nc


def _host_bias(abs_pos: np.ndarray) -> np.ndarray:
    pos = np.asarray(abs_pos, dtype=np.float32)[0, 0, :, :, 0, 0]  # [C, PN]
    idx = (np.arange(PN)[None, :] + np.arange(C)[:, None]) % PN    # [C, p] -> q
    bias = np.take_along_axis(pos, idx, axis=1)                    # [C, p]
    # channel c = i + 32*c2 -> row block c2, column block i
    bias = bias.reshape(2, NPAIR, PN)                              # [c2, i, p]
    bias = bias.reshape(2, NPAIR * PN)                             # [c2, i*64+p]
    bias = np.repeat(bias, 64, axis=0)                             # [128, ...]
    return np.ascontiguousarray(bias)


def _interleave(xb: np.ndarray) -> np.ndarray:
    # [C, H, W] -> [NPAIR, 128, FD] tile layout; c2=1 rows pre-rotated by
    # half the free dim (qi -> (qi+4) % 8) so both halves share one shift.
    v = xb.reshape(2, NPAIR, 8, 64, 8, 64)         # (c2, i, qi, a, qj, d)
    v = np.concatenate([v[:1], np.roll(v[1:], -4, axis=2)], axis=0)
    v = v.transpose(1, 0, 3, 2, 4, 5)              # (i, c2, a, qi, qj, d)
    return np.ascontiguousarray(v.reshape(NPAIR, 128, FD))


def _deinterleave(ob: np.ndarray) -> np.ndarray:
    # [NPAIR, 128, FD] (true p-order for both halves) -> [C, H, W]
    v = ob.reshape(NPAIR, 2, 64, 8, 8, 64)         # (i, c2, a, pi, pj, d)
    v = v.transpose(1, 0, 3, 2, 4, 5)              # (c2, i, pi, a, pj, d)
    return v.reshape(C, H, W)


def kernel(x: np.ndarray, abs_pos: np.ndarray) -> np.ndarray:
    global LAST_RESULTS
    x = np.asarray(x, dtype=np.float32)
    assert x.shape == (B, C, H, W), x.shape

    bias = _host_bias(abs_pos)

    if "nc" not in _NC_CACHE:
        _NC_CACHE["nc"] = _build_nc()
    nc = _NC_CACHE["nc"]

    in_maps = [{"x": _interleave(x[b]), "bias": bias} for b in range(B)]
    res = run_bass_kernel_spmd(
        nc,
        in_maps,
        core_ids=list(range(B)),
        trace=bool(os.environ.get("KERNEL_TRACE")),
    )
    LAST_RESULTS = res
    return np.stack(
        [_deinterleave(res.results[b]["out"]) for b in range(B)], axis=0
    )
